# revision 17
# baseline (speedup 1.0000x reference)
"""Trainium2 Bass kernel for nn_Attention (B=2, S=2048, D=1024, H=16, causal).

Sharding: head-parallel across 8 NeuronCores — 2 heads per core. Each core:
  1. computes qT/kT/vT for its 2 heads from the full xT (QKV projection,
     transposed layout [128 = 2*hd, S]),
  2. runs causal attention per head with scores in transposed orientation
     (sT[sj, si]) so the PV matmul needs no P transpose; the softmax
     denominator comes free as an extra ones-column in the V operand,
  3. multiplies by its 128-row slice of W_proj producing a partial output
     yT_c [B, D, S].
Host sums the 8 partials, adds b_proj, and transposes back to [B, S, D].

All matmuls run in float32r (full-rate fp32 on the PE; ~1e-4 rounding).
"""
import sys

sys.path.insert(0, "/opt/trn_rl_repo")

import numpy as np
import concourse.bacc as bacc
import concourse.mybir as mybir
import concourse.tile as tile
from concourse.bass_utils import run_bass_kernel_spmd

dt = mybir.dt
F32R = dt.float32r
AF = mybir.ActivationFunctionType

B, S, D, H = 2, 2048, 1024, 16
HD = D // H            # 64
NCORE = 8
HPC = H // NCORE       # 2 heads per core
NEG = -30000.0         # exp((s + NEG) * 0.125) == 0 in fp32

_CACHE = {}


def build_nc():
    nc = bacc.Bacc("TRN2", target_bir_lowering=False, debug=False)

    xT_d = nc.dram_tensor("xT", [B, D, S], dt.float32, kind="ExternalInput")
    wq_d = nc.dram_tensor("wq", [D, 128], dt.float32, kind="ExternalInput")
    wk_d = nc.dram_tensor("wk", [D, 128], dt.float32, kind="ExternalInput")
    wv_d = nc.dram_tensor("wv", [D, 128], dt.float32, kind="ExternalInput")
    bq_d = nc.dram_tensor("bq", [128, 1], dt.float32, kind="ExternalInput")
    bk_d = nc.dram_tensor("bk", [128, 1], dt.float32, kind="ExternalInput")
    bv_d = nc.dram_tensor("bv", [128, 1], dt.float32, kind="ExternalInput")
    wp_d = nc.dram_tensor("wp", [128, D], dt.float32, kind="ExternalInput")
    negm_d = nc.dram_tensor("negm2", [128, 256], dt.float32, kind="ExternalInput")
    id_d = nc.dram_tensor("ident", [128, 128], dt.float32, kind="ExternalInput")
    ones_d = nc.dram_tensor("ones", [128, 64], dt.float32, kind="ExternalInput")
    zer_d = nc.dram_tensor("zer", [64, S], dt.float32, kind="ExternalInput")
    yT_d = nc.dram_tensor("yT", [B, D, S], dt.float32, kind="ExternalOutput")

    with tile.TileContext(nc) as tc:
        with (
            tc.tile_pool(name="consts", bufs=1) as consts,
            tc.tile_pool(name="xpool", bufs=1) as xpool,
            tc.tile_pool(name="vpool", bufs=1) as vpool,
            tc.tile_pool(name="qkv", bufs=2) as qkvp,
            tc.tile_pool(name="epool", bufs=3) as epool,
            tc.tile_pool(name="ypool", bufs=6) as ypool,
            tc.tile_pool(name="rpool", bufs=2) as rpool,
            # PSUM: pair tiles [128,2,512] = 2 banks x 2 bufs = 4 banks;
            # psa [65,512] x 2 bufs = 2; aux (proj/transpose/bcast) x 2 = 2.
            tc.tile_pool(name="ps_mm2", bufs=2, space="PSUM") as ps_mm2,
            tc.tile_pool(name="ps_a", bufs=2, space="PSUM") as ps_a_pool,
            tc.tile_pool(name="ps_aux", bufs=2, space="PSUM") as ps_aux,
        ):
            # ---- constants / weights (once) ----
            wqr = consts.tile([128, 8, 128], F32R, tag="wq")
            wkr = consts.tile([128, 8, 128], F32R, tag="wk")
            wvr = consts.tile([128, 8, 128], F32R, tag="wv")
            for (w_r, w_d) in ((wqr, wq_d), (wkr, wk_d), (wvr, wv_d)):
                for d in range(8):
                    nc.gpsimd.dma_start(
                        w_r[:, d, :], w_d.ap()[128 * d:128 * (d + 1), :]
                    )
            wpr = consts.tile([128, D], F32R, tag="wp")
            nc.gpsimd.dma_start(wpr[:], wp_d.ap()[:])
            bq_sb = consts.tile([128, 1], dt.float32, tag="bq")
            bk_sb = consts.tile([128, 1], dt.float32, tag="bk")
            bv_sb = consts.tile([128, 1], dt.float32, tag="bv")
            nc.sync.dma_start(bq_sb[:], bq_d.ap()[:])
            nc.sync.dma_start(bk_sb[:], bk_d.ap()[:])
            nc.sync.dma_start(bv_sb[:], bv_d.ap()[:])
            negm2 = consts.tile([128, 2, 128], dt.float32, tag="negm2")
            nc.sync.dma_start(negm2[:], negm_d.ap().rearrange("p (t f) -> p t f", t=2))
            ident = consts.tile([128, 128], dt.float32, tag="ident")
            nc.sync.dma_start(ident[:], id_d.ap()[:])
            ones_r = consts.tile([128, 64], F32R, tag="ones")
            nc.gpsimd.dma_start(ones_r[:], ones_d.ap()[:, :])

            def emit_proj(b, blk, aT):
                si0 = 512 * blk
                for dtile in range(8):
                    ps = ps_aux.tile([128, 512], dt.float32, tag="aux",
                                     name=f"psp_{b}_{blk}_{dtile}")
                    nc.tensor.matmul(
                        ps[:],
                        wpr[:, 128 * dtile:128 * (dtile + 1)],
                        aT[:, si0:si0 + 512],
                        start=True,
                        stop=True,
                    )
                    y_sb = ypool.tile([128, 512], dt.float32, tag="y",
                                      name=f"y_{b}_{blk}_{dtile}")
                    nc.vector.tensor_copy(y_sb[:], ps[:])
                    dma_eng = nc.sync if dtile % 2 == 0 else nc.scalar
                    dma_eng.dma_start(
                        yT_d.ap()[
                            b, 128 * dtile:128 * (dtile + 1), si0:si0 + 512,
                        ],
                        y_sb[:],
                    )

            for b in range(B):
                # ---- load xT[b] as f32r, per d-tile ----
                xr = xpool.tile([128, 8, S], F32R, tag="x")
                for d in range(8):
                    nc.gpsimd.dma_start(
                        xr[:, d, :], xT_d.ap()[b, 128 * d:128 * (d + 1), :]
                    )

                # ---- QKV projection (transposed outputs) ----
                # kT split per head into zero-padded [128, S] tensors so the
                # score matmuls contract over the full K=128 array (the clock
                # gate treats half-array matmuls as idle).
                qTr = qkvp.tile([128, S], F32R, tag="qT")
                kp0 = qkvp.tile([128, S], F32R, tag="kp0")
                kp1 = qkvp.tile([128, S], F32R, tag="kp1")
                nc.gpsimd.dma_start(kp0[64:128, :], zer_d.ap()[:])
                nc.gpsimd.dma_start(kp1[0:64, :], zer_d.ap()[:])
                vT = vpool.tile([128, S], dt.float32, tag="vT")
                for (w_r, bias, kind) in (
                    (wqr, bq_sb, "q"),
                    (wkr, bk_sb, "k"),
                    (wvr, bv_sb, "v"),
                ):
                    for bp in range(2):  # pairs of 512-blocks
                        pp = ps_mm2.tile([128, 2, 512], dt.float32, tag="mm2",
                                         name=f"qkv_{b}_{kind}_{bp}")
                        for d in range(8):
                            for t in range(2):
                                blk = 2 * bp + t
                                nc.tensor.matmul(
                                    pp[:, t, :],
                                    w_r[:, d, :],
                                    xr[:, d, 512 * blk:512 * (blk + 1)],
                                    start=(d == 0),
                                    stop=(d == 7),
                                )
                        cols = slice(1024 * bp, 1024 * (bp + 1))
                        src = pp[:].rearrange("p t f -> p (t f)")
                        if kind == "q":
                            nc.scalar.activation(qTr[:, cols], src,
                                                 AF.Identity, bias=bias[:, 0:1])
                        elif kind == "v":
                            nc.scalar.activation(vT[:, cols], src,
                                                 AF.Identity, bias=bias[:, 0:1])
                        else:
                            nc.scalar.activation(kp0[0:64, cols], src[0:64, :],
                                                 AF.Identity, bias=bias[0:64, 0:1])
                            nc.scalar.activation(kp1[64:128, cols], src[64:128, :],
                                                 AF.Identity, bias=bias[64:128, 0:1])

                # ---- vhat: v natural per sj tile + ones column, f32r ----
                vhat = qkvp.tile([128, 16, 130], F32R, tag="vhat")
                nc.gpsimd.dma_start(vhat[:, :, 64], ones_d.ap()[:, 0:16])
                nc.gpsimd.dma_start(vhat[:, :, 129], ones_d.ap()[:, 16:32])
                for j in range(16):
                    pst = ps_aux.tile([128, 128], dt.float32, tag="aux",
                                      name=f"tr_{b}_{j}")
                    nc.tensor.transpose(
                        pst[:], vT[:, 128 * j:128 * (j + 1)], ident[:]
                    )
                    nc.vector.tensor_copy(vhat[:, j, 0:64], pst[:, 0:64])
                    nc.vector.tensor_copy(vhat[:, j, 65:129], pst[:, 64:128])

                # ---- causal attention, heads paired in adjacent PSUM banks --
                aT = qkvp.tile([128, S], F32R, tag="aT")
                for blk in range(4):
                    si0 = 512 * blk
                    jlast = 4 * blk + 3
                    psa = [
                        ps_a_pool.tile([65, 512], dt.float32, tag="acc",
                                       name=f"psa_{b}_{blk}_{hl}")
                        for hl in range(HPC)
                    ]
                    for j in range(jlast + 1):
                        off = max(0, 128 * (j - 4 * blk))
                        w = 512 - off
                        pp = ps_mm2.tile([128, 2, 512], dt.float32, tag="mm2",
                                         name=f"pp_{b}_{blk}_{j}")
                        for hl, kp in ((0, kp0), (1, kp1)):
                            nc.tensor.matmul(
                                pp[:, hl, 0:w],
                                kp[:, 128 * j:128 * (j + 1)],
                                qTr[:, si0 + off:si0 + 512],
                                start=True,
                                stop=True,
                            )
                        if j >= 4 * blk:
                            nc.vector.tensor_add(
                                pp[:, :, 0:128], pp[:, :, 0:128], negm2[:]
                            )
                        ee = epool.tile([128, 2, 512], F32R, tag="eT",
                                        name=f"ee_{b}_{blk}_{j}")
                        nc.scalar.activation(
                            ee[:, :, 0:w], pp[:, :, 0:w], AF.Exp, scale=0.125
                        )
                        for hl in range(HPC):
                            nc.tensor.matmul(
                                psa[hl][:, off:512],
                                vhat[:, j, 65 * hl:65 * hl + 65],
                                ee[:, hl, 0:w],
                                start=(j == 0),
                                stop=(j == jlast),
                            )
                    if blk > 0:
                        emit_proj(b, blk - 1, aT)
                    for hl in range(HPC):
                        p0 = 64 * hl
                        # drain psa to SBUF early (frees the PSUM slot), then
                        # normalize: broadcast l via K=1 matmul, reciprocal on
                        # the broadcast, multiply.
                        a_sb = rpool.tile([65, 512], F32R, tag="a_sb",
                                          name=f"asb_{b}_{blk}_{hl}")
                        nc.scalar.activation(a_sb[:], psa[hl][:], AF.Identity)
                        lnl = rpool.tile([1, 512], F32R, tag="lnl",
                                         name=f"lnl_{b}_{blk}_{hl}")
                        nc.scalar.activation(lnl[:], psa[hl][64:65, :], AF.Ln)
                        psb = ps_aux.tile([64, 512], dt.float32, tag="aux",
                                          name=f"psb_{b}_{blk}_{hl}")
                        nc.tensor.matmul(
                            psb[:], ones_r[0:1, :], lnl[:],
                            start=True, stop=True
                        )
                        rec_sb = rpool.tile([64, 512], dt.float32, tag="rec_sb",
                                            name=f"recs_{b}_{blk}_{hl}")
                        nc.scalar.activation(rec_sb[:], psb[:], AF.Exp,
                                             scale=-1.0)
                        with nc.allow_low_precision(reason="f32r attn normalize"):
                            nc.vector.tensor_mul(
                                aT[p0:p0 + 64, si0:si0 + 512],
                                a_sb[0:64, :],
                                rec_sb[:],
                            )
                emit_proj(b, 3, aT)
    nc.compile()
    return nc


def _get_nc():
    if "nc" not in _CACHE:
        _CACHE["nc"] = build_nc()
    return _CACHE["nc"]


def make_in_maps(x, W_attn, b_attn, W_proj):
    x = np.ascontiguousarray(x, dtype=np.float32)
    xT = np.ascontiguousarray(x.transpose(0, 2, 1))

    p = np.arange(128)
    negm = np.where(p[:, None] <= p[None, :], 0.0, NEG).astype(np.float32)
    negm2 = np.concatenate([negm, negm], axis=1)
    ident = np.eye(128, dtype=np.float32)
    ones = np.ones((128, 64), np.float32)

    in_maps = []
    for c in range(NCORE):
        col0 = HD * HPC * c
        in_maps.append({
            "xT": xT,
            "wq": np.ascontiguousarray(W_attn[:, col0:col0 + 128]),
            "wk": np.ascontiguousarray(W_attn[:, D + col0:D + col0 + 128]),
            "wv": np.ascontiguousarray(W_attn[:, 2 * D + col0:2 * D + col0 + 128]),
            "bq": np.ascontiguousarray(b_attn[col0:col0 + 128].reshape(128, 1)),
            "bk": np.ascontiguousarray(b_attn[D + col0:D + col0 + 128].reshape(128, 1)),
            "bv": np.ascontiguousarray(b_attn[2 * D + col0:2 * D + col0 + 128].reshape(128, 1)),
            "wp": np.ascontiguousarray(W_proj[128 * c:128 * (c + 1), :]),
            "negm2": negm2,
            "zer": np.zeros((64, S), np.float32),
            "ident": ident,
            "ones": ones,
        })
    return in_maps


def gather(results, b_proj):
    acc = np.zeros((B, D, S), np.float64)
    for r in results:
        acc += r["yT"]
    out = acc.transpose(0, 2, 1) + np.asarray(b_proj, np.float64)[None, None, :]
    return np.ascontiguousarray(out.astype(np.float32))


def kernel(x, W_attn, b_attn, W_proj, b_proj, _trace=False, _trace_kwargs=None):
    nc = _get_nc()
    in_maps = make_in_maps(np.asarray(x), np.asarray(W_attn),
                           np.asarray(b_attn), np.asarray(W_proj))
    res = run_bass_kernel_spmd(
        nc, in_maps, list(range(NCORE)), trace=_trace, **(_trace_kwargs or {})
    )
    out = gather(res.results, np.asarray(b_proj))
    if _trace:
        kernel.last_result = res
    return out


# revision 18
# speedup vs baseline: 1.1003x; 1.1003x over previous
"""Trainium2 Bass kernel for nn_Attention (B=2, S=2048, D=1024, H=16, causal).

Sharding: head-parallel across 8 NeuronCores — 2 heads per core. Each core:
  1. computes qT/kT/vT for its 2 heads from the full xT (QKV projection,
     transposed layout [128 = 2*hd, S]),
  2. runs causal attention per head with scores in transposed orientation
     (sT[sj, si]) so the PV matmul needs no P transpose; the softmax
     denominator comes free as an extra ones-column in the V operand,
  3. multiplies by its 128-row slice of W_proj producing a partial output
     yT_c [B, D, S].
Host sums the 8 partials, adds b_proj, and transposes back to [B, S, D].

All matmuls run in float32r (full-rate fp32 on the PE; ~1e-4 rounding).
"""
import sys

sys.path.insert(0, "/opt/trn_rl_repo")

import numpy as np
import concourse.bacc as bacc
import concourse.mybir as mybir
import concourse.tile as tile
from concourse.bass_utils import run_bass_kernel_spmd

dt = mybir.dt
F32R = dt.float32r
AF = mybir.ActivationFunctionType

B, S, D, H = 2, 2048, 1024, 16
HD = D // H            # 64
NCORE = 8
HPC = H // NCORE       # 2 heads per core
NEG = -30000.0         # exp((s + NEG) * 0.125) == 0 in fp32

_CACHE = {}


def build_nc():
    nc = bacc.Bacc("TRN2", target_bir_lowering=False, debug=False)

    xT_d = nc.dram_tensor("xT", [B, D, S], dt.float32, kind="ExternalInput")
    wq_d = nc.dram_tensor("wq", [D, 128], dt.float32, kind="ExternalInput")
    wk_d = nc.dram_tensor("wk", [D, 128], dt.float32, kind="ExternalInput")
    wv_d = nc.dram_tensor("wv", [D, 128], dt.float32, kind="ExternalInput")
    bq_d = nc.dram_tensor("bq", [128, 1], dt.float32, kind="ExternalInput")
    bk_d = nc.dram_tensor("bk", [128, 1], dt.float32, kind="ExternalInput")
    bv_d = nc.dram_tensor("bv", [128, 1], dt.float32, kind="ExternalInput")
    wp_d = nc.dram_tensor("wp", [128, D], dt.float32, kind="ExternalInput")
    negm_d = nc.dram_tensor("negm2", [128, 256], dt.float32, kind="ExternalInput")
    id_d = nc.dram_tensor("ident", [128, 128], dt.float32, kind="ExternalInput")
    ones_d = nc.dram_tensor("ones", [128, 64], dt.float32, kind="ExternalInput")
    zer_d = nc.dram_tensor("zer", [64, S], dt.float32, kind="ExternalInput")
    yT_d = nc.dram_tensor("yT", [B, D, S], dt.float32, kind="ExternalOutput")

    with tile.TileContext(nc) as tc:
        with (
            tc.tile_pool(name="consts", bufs=1) as consts,
            tc.tile_pool(name="xpool", bufs=1) as xpool,
            tc.tile_pool(name="vpool", bufs=1) as vpool,
            tc.tile_pool(name="qkv", bufs=2) as qkvp,
            tc.tile_pool(name="epool", bufs=2) as epool,
            tc.tile_pool(name="ypool", bufs=4) as ypool,
            tc.tile_pool(name="rpool", bufs=3) as rpool,
            # PSUM: pair tiles [128,2,512] = 2 banks x 2 bufs = 4 banks;
            # psa [65,512] x 2 bufs = 2; aux (proj/transpose/bcast) x 2 = 2.
            tc.tile_pool(name="ps_mm2", bufs=2, space="PSUM") as ps_mm2,
            tc.tile_pool(name="ps_a", bufs=2, space="PSUM") as ps_a_pool,
            tc.tile_pool(name="ps_aux", bufs=2, space="PSUM") as ps_aux,
        ):
            # ---- constants / weights (once) ----
            wqr = consts.tile([128, 8, 128], F32R, tag="wq")
            wkr = consts.tile([128, 8, 128], F32R, tag="wk")
            wvr = consts.tile([128, 8, 128], F32R, tag="wv")
            for (w_r, w_d) in ((wqr, wq_d), (wkr, wk_d), (wvr, wv_d)):
                for d in range(8):
                    nc.gpsimd.dma_start(
                        w_r[:, d, :], w_d.ap()[128 * d:128 * (d + 1), :]
                    )
            wpr = consts.tile([128, D], F32R, tag="wp")
            nc.gpsimd.dma_start(wpr[:], wp_d.ap()[:])
            bq_sb = consts.tile([128, 1], dt.float32, tag="bq")
            bk_sb = consts.tile([128, 1], dt.float32, tag="bk")
            bv_sb = consts.tile([128, 1], dt.float32, tag="bv")
            nc.sync.dma_start(bq_sb[:], bq_d.ap()[:])
            nc.sync.dma_start(bk_sb[:], bk_d.ap()[:])
            nc.sync.dma_start(bv_sb[:], bv_d.ap()[:])
            negm2 = consts.tile([128, 2, 128], dt.float32, tag="negm2")
            nc.sync.dma_start(negm2[:], negm_d.ap().rearrange("p (t f) -> p t f", t=2))
            ident = consts.tile([128, 128], dt.float32, tag="ident")
            nc.sync.dma_start(ident[:], id_d.ap()[:])
            ones_r = consts.tile([128, 64], F32R, tag="ones")
            nc.gpsimd.dma_start(ones_r[:], ones_d.ap()[:, :])

            def emit_proj(b, blk, aT):
                si0 = 512 * blk
                for dtile in range(8):
                    ps = ps_aux.tile([128, 512], dt.float32, tag="aux",
                                     name=f"psp_{b}_{blk}_{dtile}")
                    nc.tensor.matmul(
                        ps[:],
                        wpr[:, 128 * dtile:128 * (dtile + 1)],
                        aT[:, si0:si0 + 512],
                        start=True,
                        stop=True,
                    )
                    y_sb = ypool.tile([128, 512], dt.float32, tag="y",
                                      name=f"y_{b}_{blk}_{dtile}")
                    nc.vector.tensor_copy(y_sb[:], ps[:])
                    dma_eng = nc.sync if dtile % 2 == 0 else nc.scalar
                    dma_eng.dma_start(
                        yT_d.ap()[
                            b, 128 * dtile:128 * (dtile + 1), si0:si0 + 512,
                        ],
                        y_sb[:],
                    )

            for b in range(B):
                # ---- load xT[b] as f32r, per d-tile ----
                xr = xpool.tile([128, 8, S], F32R, tag="x")
                for d in range(8):
                    nc.gpsimd.dma_start(
                        xr[:, d, :], xT_d.ap()[b, 128 * d:128 * (d + 1), :]
                    )

                # ---- QKV projection (transposed outputs) ----
                # kT split per head into zero-padded [128, S] tensors so the
                # score matmuls contract over the full K=128 array (the clock
                # gate treats half-array matmuls as idle).
                qTr = qkvp.tile([128, S], F32R, tag="qT")
                kp0 = qkvp.tile([128, S], F32R, tag="kp0")
                kp1 = qkvp.tile([128, S], F32R, tag="kp1")
                nc.gpsimd.dma_start(kp0[64:128, :], zer_d.ap()[:])
                nc.gpsimd.dma_start(kp1[0:64, :], zer_d.ap()[:])
                vT = vpool.tile([128, S], dt.float32, tag="vT")
                for (w_r, bias, kind) in (
                    (wqr, bq_sb, "q"),
                    (wkr, bk_sb, "k"),
                    (wvr, bv_sb, "v"),
                ):
                    for bp in range(2):  # pairs of 512-blocks
                        pp = ps_mm2.tile([128, 2, 512], dt.float32, tag="mm2",
                                         name=f"qkv_{b}_{kind}_{bp}")
                        for d in range(8):
                            for t in range(2):
                                blk = 2 * bp + t
                                nc.tensor.matmul(
                                    pp[:, t, :],
                                    w_r[:, d, :],
                                    xr[:, d, 512 * blk:512 * (blk + 1)],
                                    start=(d == 0),
                                    stop=(d == 7),
                                )
                        cols = slice(1024 * bp, 1024 * (bp + 1))
                        src = pp[:].rearrange("p t f -> p (t f)")
                        if kind == "q":
                            nc.scalar.activation(qTr[:, cols], src,
                                                 AF.Identity, bias=bias[:, 0:1])
                        elif kind == "v":
                            nc.scalar.activation(vT[:, cols], src,
                                                 AF.Identity, bias=bias[:, 0:1])
                        else:
                            nc.scalar.activation(kp0[0:64, cols], src[0:64, :],
                                                 AF.Identity, bias=bias[0:64, 0:1])
                            nc.scalar.activation(kp1[64:128, cols], src[64:128, :],
                                                 AF.Identity, bias=bias[64:128, 0:1])

                # ---- vhat: v natural per sj tile + ones column, f32r ----
                vhat = qkvp.tile([128, 16, 130], F32R, tag="vhat")
                nc.gpsimd.dma_start(vhat[:, :, 64], ones_d.ap()[:, 0:16])
                nc.gpsimd.dma_start(vhat[:, :, 129], ones_d.ap()[:, 16:32])
                for j in range(16):
                    pst = ps_aux.tile([128, 128], dt.float32, tag="aux",
                                      name=f"tr_{b}_{j}")
                    nc.tensor.transpose(
                        pst[:], vT[:, 128 * j:128 * (j + 1)], ident[:]
                    )
                    nc.vector.tensor_copy(vhat[:, j, 0:64], pst[:, 0:64])
                    nc.vector.tensor_copy(vhat[:, j, 65:129], pst[:, 64:128])

                # ---- causal attention, heads paired in adjacent PSUM banks --
                aT = qkvp.tile([128, S], F32R, tag="aT")
                for blk in range(4):
                    si0 = 512 * blk
                    jlast = 4 * blk + 3
                    psa = [
                        ps_a_pool.tile([65, 512], dt.float32, tag="acc",
                                       name=f"psa_{b}_{blk}_{hl}")
                        for hl in range(HPC)
                    ]
                    for j in range(jlast + 1):
                        off = max(0, 128 * (j - 4 * blk))
                        w = 512 - off
                        pp = ps_mm2.tile([128, 2, 512], dt.float32, tag="mm2",
                                         name=f"pp_{b}_{blk}_{j}")
                        for hl, kp in ((0, kp0), (1, kp1)):
                            nc.tensor.matmul(
                                pp[:, hl, 0:w],
                                kp[:, 128 * j:128 * (j + 1)],
                                qTr[:, si0 + off:si0 + 512],
                                start=True,
                                stop=True,
                            )
                        if j >= 4 * blk:
                            nc.vector.tensor_add(
                                pp[:, :, 0:128], pp[:, :, 0:128], negm2[:]
                            )
                        ee = epool.tile([128, 2, 512], F32R, tag="eT",
                                        name=f"ee_{b}_{blk}_{j}")
                        nc.scalar.activation(
                            ee[:, :, 0:w], pp[:, :, 0:w], AF.Exp, scale=0.125
                        )
                        for hl in range(HPC):
                            nc.tensor.matmul(
                                psa[hl][:, off:512],
                                vhat[:, j, 65 * hl:65 * hl + 65],
                                ee[:, hl, 0:w],
                                start=(j == 0),
                                stop=(j == jlast),
                            )
                    if blk > 0:
                        emit_proj(b, blk - 1, aT)
                    for hl in range(HPC):
                        p0 = 64 * hl
                        # drain psa to SBUF early (frees the PSUM slot), then
                        # normalize: broadcast l via K=1 matmul, reciprocal on
                        # the broadcast, multiply.
                        a_sb = rpool.tile([65, 512], F32R, tag="a_sb",
                                          name=f"asb_{b}_{blk}_{hl}")
                        nc.scalar.activation(a_sb[:], psa[hl][:], AF.Identity)
                        lnl = rpool.tile([1, 512], F32R, tag="lnl",
                                         name=f"lnl_{b}_{blk}_{hl}")
                        nc.scalar.activation(lnl[:], psa[hl][64:65, :], AF.Ln)
                        psb = ps_aux.tile([64, 512], dt.float32, tag="aux",
                                          name=f"psb_{b}_{blk}_{hl}")
                        nc.tensor.matmul(
                            psb[:], ones_r[0:1, :], lnl[:],
                            start=True, stop=True
                        )
                        rec_sb = rpool.tile([64, 512], dt.float32, tag="rec_sb",
                                            name=f"recs_{b}_{blk}_{hl}")
                        nc.scalar.activation(rec_sb[:], psb[:], AF.Exp,
                                             scale=-1.0)
                        with nc.allow_low_precision(reason="f32r attn normalize"):
                            nc.vector.tensor_mul(
                                aT[p0:p0 + 64, si0:si0 + 512],
                                a_sb[0:64, :],
                                rec_sb[:],
                            )
                emit_proj(b, 3, aT)
    nc.compile()
    return nc


def _get_nc():
    if "nc" not in _CACHE:
        _CACHE["nc"] = build_nc()
    return _CACHE["nc"]


def make_in_maps(x, W_attn, b_attn, W_proj):
    x = np.ascontiguousarray(x, dtype=np.float32)
    xT = np.ascontiguousarray(x.transpose(0, 2, 1))

    p = np.arange(128)
    negm = np.where(p[:, None] <= p[None, :], 0.0, NEG).astype(np.float32)
    negm2 = np.concatenate([negm, negm], axis=1)
    ident = np.eye(128, dtype=np.float32)
    ones = np.ones((128, 64), np.float32)

    in_maps = []
    for c in range(NCORE):
        col0 = HD * HPC * c
        in_maps.append({
            "xT": xT,
            "wq": np.ascontiguousarray(W_attn[:, col0:col0 + 128]),
            "wk": np.ascontiguousarray(W_attn[:, D + col0:D + col0 + 128]),
            "wv": np.ascontiguousarray(W_attn[:, 2 * D + col0:2 * D + col0 + 128]),
            "bq": np.ascontiguousarray(b_attn[col0:col0 + 128].reshape(128, 1)),
            "bk": np.ascontiguousarray(b_attn[D + col0:D + col0 + 128].reshape(128, 1)),
            "bv": np.ascontiguousarray(b_attn[2 * D + col0:2 * D + col0 + 128].reshape(128, 1)),
            "wp": np.ascontiguousarray(W_proj[128 * c:128 * (c + 1), :]),
            "negm2": negm2,
            "zer": np.zeros((64, S), np.float32),
            "ident": ident,
            "ones": ones,
        })
    return in_maps


def gather(results, b_proj):
    acc = np.zeros((B, D, S), np.float64)
    for r in results:
        acc += r["yT"]
    out = acc.transpose(0, 2, 1) + np.asarray(b_proj, np.float64)[None, None, :]
    return np.ascontiguousarray(out.astype(np.float32))


def kernel(x, W_attn, b_attn, W_proj, b_proj, _trace=False, _trace_kwargs=None):
    nc = _get_nc()
    in_maps = make_in_maps(np.asarray(x), np.asarray(W_attn),
                           np.asarray(b_attn), np.asarray(W_proj))
    res = run_bass_kernel_spmd(
        nc, in_maps, list(range(NCORE)), trace=_trace, **(_trace_kwargs or {})
    )
    out = gather(res.results, np.asarray(b_proj))
    if _trace:
        kernel.last_result = res
    return out
